# revision 45
# baseline (speedup 1.0000x reference)
"""CenterLoss (segment_reduce) Trainium2 kernel — fp8, engine-balanced f2.

Data-parallel over N across 8 cores; all feature traffic in fp8 e4m3
(rel-err budget 2e-2; measured error ~1e-4 level).

Host prep (layout/dtype only):
  natf : [NGRP*128, GRP*512] fp8, group-major: row (g, p) holds the 16
         u-slots of group g back to back (u, t, d), 8KB contiguous per
         partition per group DMA.
  tgp  : [128, U*2] fp8 class ids, tgp[p, (u,t)] = target(row(u,p,t)).
  fta/ftb : [128, R] fp8 transposed feature halves (d major) for pass 2.

Device (per rep):
  oh8 one-hot built on DVE from tgp (8 is_equal ops).
  Pass 1 per group: DoubleRow matmuls oh.T @ f accumulate class sums
    [8,256] in PSUM; counts via ones.T @ oh.  f2 (row norms) is computed
    entirely from the natural stage tiles, batched per engine to amortize
    the fixed per-instruction overheads: DVE / ACT / POOL each square a
    contiguous u-range of the group (scrap bf16), reduces on DVE/POOL.
  AllReduce [8, 257] sums+counts (ncfw, gpsimd bounces).
  centers, W = -2*S*centers.T in fp8, c2' = S*c2 + 240*empty.
  Pass 2 per block: rank-1 c2' matmul + 128 FWL matmuls into a PSUM
    bank; DVE min over classes; d2 = mn/S + f2; ACT sqrt with per-block
    accum column; final ones matmul -> scalar.
"""

import numpy as np

from concourse import bass, bacc, mybir, tile
from concourse import bass_utils

F32 = mybir.dt.float32
BF16 = mybir.dt.bfloat16
FP8 = mybir.dt.float8e4
OP = mybir.AluOpType
AFT = mybir.ActivationFunctionType
PM = mybir.MatmulPerfMode

N_TOTAL = 524288
D = 256
C = 8
NCORES = 8
P = 128

LINE = 256            # per-t bytes (one-hot lives in oh8)
GRP = 16              # u-slots per stage group
CPB = 64              # chunks per PSUM bank / block
S = 256.0             # fp8 scale for W / c2
BIGC = 240.0          # empty-class c2 mask (fp8 max finite)

import os
DBG = set(os.environ.get("KDBG", "").split(","))

# f2 engine split: u-slots per group handled by DVE / POOL / ACT.
# DVE: batched square+reduce on a separate bf16 copy of the features
# (bf16 gets the 2x DVE mode; DMA is cheap, DVE cycles are not);
# POOL: none by default (gpsimd shares SBUF ports with DVE and its
# 2-input ops are slow); ACT: per-chunk Square with accum_out.
NDV = int(os.environ.get("KNDV", "4"))
NPL = int(os.environ.get("KNPL", "8"))
NAC = GRP - NDV - NPL
B16 = "b16" in DBG


def build_nc(R: int, reps: int = 1):
    assert R % 256 == 0
    U = R // 256          # u-slots (256 rows each)
    NCHUNK = R // P       # 128-row chunks
    NBLK = NCHUNK // CPB
    NGRP = U // GRP

    nc = bacc.Bacc(
        "TRN2", target_bir_lowering=False, debug=False, num_devices=NCORES
    )
    nat_t = nc.dram_tensor(
        "natf", [(U // GRP) * P, GRP * 2 * LINE], FP8, kind="ExternalInput"
    )
    natb_t = None
    if B16:
        natb_t = nc.dram_tensor(
            "natb", [(U // GRP) * P, GRP * 2 * D], BF16,
            kind="ExternalInput",
        )
    tgp_t = nc.dram_tensor("tgp", [P, U * 2], FP8, kind="ExternalInput")
    fta_t = nc.dram_tensor("fta", [P, R], FP8, kind="ExternalInput")
    ftb_t = nc.dram_tensor("ftb", [P, R], FP8, kind="ExternalInput")
    out_t = nc.dram_tensor("partial", [reps, 1], F32, kind="ExternalOutput")

    with tile.TileContext(nc) as tc:
        with (
            tc.tile_pool(name="const", bufs=1) as constp,
            tc.tile_pool(name="stage", bufs=2) as stagep,
            tc.tile_pool(name="tg", bufs=2) as tgp,
            tc.tile_pool(name="oh", bufs=1) as ohp,
            tc.tile_pool(name="ft", bufs=NBLK) as ftp,
            tc.tile_pool(name="scr", bufs=2) as scrp,
            tc.tile_pool(name="f2", bufs=NBLK) as f2p,
            tc.tile_pool(name="blk", bufs=2) as blkp,
            tc.tile_pool(name="small", bufs=1) as smallp,
            tc.tile_pool(name="ps_acc", bufs=1, space="PSUM") as ps_accp,
            tc.tile_pool(name="ps_cnt", bufs=1, space="PSUM") as ps_cntp,
            tc.tile_pool(name="ps_fc", bufs=2, space="PSUM") as ps_fcp,
            tc.tile_pool(name="ps_small", bufs=2, space="PSUM") as ps_smallp,
            tc.tile_pool(name="dram", bufs=1, space="DRAM") as dramp,
        ):
            # ---------------- constants ----------------
            ones_c8 = constp.tile([P, 1], FP8)
            nc.vector.memset(ones_c8[:], 1.0)
            ones_r8 = constp.tile([1, P], FP8)
            nc.vector.memset(ones_r8[:], 1.0)
            ones_c32 = constp.tile([P, 1], F32)
            nc.vector.memset(ones_c32[:], 1.0)

            cls8 = constp.tile([C, C], F32)
            for c in range(C):
                nc.vector.memset(cls8[:, c : c + 1], float(c))
            pidx_i = constp.tile([C, 1], mybir.dt.int32)
            nc.gpsimd.iota(pidx_i[:], pattern=[[0, 1]], base=0,
                           channel_multiplier=1)
            pidx = constp.tile([C, 1], F32)
            nc.vector.tensor_copy(pidx[:], pidx_i[:])
            ident8 = constp.tile([C, C], F32)
            nc.vector.tensor_scalar(
                ident8[:], cls8[:], pidx[:], None, op0=OP.is_equal
            )

            res_prev = None
            for rep in range(reps):
                tot_cols = smallp.tile([P, NBLK], F32, tag="totc")
                ps_sums = ps_accp.tile([C, D], F32, tag="ps_sums")
                ps_cnt = ps_cntp.tile([1, GRP * 2 * C], F32, tag="ps_cnt")

                # ---- targets + one-hot (DVE) ----
                tg = tgp.tile([P, U, 2], FP8, tag="tg")
                if res_prev is not None:
                    # "ser" timing mode: gate this rep's first DMA on the
                    # previous rep's result so reps don't pipeline.
                    nc.vector.tensor_copy(tg[0:1, 0, 0:1], res_prev[:])
                nc.sync.dma_start(
                    tg[:].rearrange("p u t -> p (u t)"), tgp_t.ap()[:, :]
                )
                oh8 = ohp.tile([P, U, 2, 16], FP8, tag="oh8")
                for c in range(C):
                    nc.vector.tensor_scalar(
                        oh8[:, :, :, c], tg[:, :, :], float(c), None,
                        op0=OP.is_equal,
                    )

                fts = {}
                f2t = {}

                # ---- pass 1 ----
                def do_group(g):
                    if g % 2 == 0:
                        f2t[g // 2] = f2p.tile(
                            [P, CPB], F32, tag="f2b", name=f"f2b_{rep}_{g//2}"
                        )
                    st = stagep.tile([P, GRP, 2, LINE], FP8, tag="stage")
                    gsl = slice(g * P, (g + 1) * P)
                    nc.sync.dma_start(
                        st[:].rearrange("p u t l -> p (u t l)"),
                        nat_t.ap()[gsl, :],
                    )
                    st16 = None
                    if natb_t is not None:
                        st16 = stagep.tile([P, GRP, 2, D], BF16, tag="st16")
                        nc.sync.dma_start(
                            st16[:].rearrange("p u t l -> p (u t l)"),
                            natb_t.ap()[gsl, :],
                        )
                    for u in range(GRP):
                        ug = g * GRP + u
                        if "nop1" in DBG:
                            break
                        nc.tensor.matmul(
                            ps_sums[:],
                            oh8[:, ug, :, 0:8],
                            st[:, u, :, :],
                            start=(ug == 0), stop=(ug == U - 1),
                            perf_mode=PM.DoubleRow,
                        )
                    if "nop1" not in DBG:
                        nc.tensor.matmul(
                            ps_cnt[:],
                            ones_c8[:],
                            oh8[:, g * GRP : (g + 1) * GRP, :, 0:8],
                            start=(g == 0), stop=(g == NGRP - 1),
                        )
                    elif g == 0:
                        nc.vector.memset(ps_sums[:], 0.0)
                        nc.vector.memset(ps_cnt[:], float(R) / (GRP * 2 * C))
                    # f2: batched square+reduce, split across DVE/ACT/POOL
                    if "nof2" in DBG:
                        if g % 2 == 0:
                            nc.vector.memset(f2t[g // 2][:], 256.0)
                        return
                    b = g // 2
                    co = (g % 2) * 2 * GRP

                    def ocol(ua, ub):
                        return (
                            f2t[b][:, co + 2 * ua : co + 2 * ub]
                            .rearrange("p (a t) -> p a t", a=ub - ua)
                        )

                    # DVE share: batched square + reduce
                    if "xdve" in DBG:
                        nc.vector.memset(
                            f2t[b][:, co : co + 2 * NDV], 256.0
                        )
                    elif st16 is not None:
                        # batched square (2x bf16 mode) in place + reduce (4x)
                        nc.vector.tensor_tensor(
                            st16[:, 0:NDV, :, :], st16[:, 0:NDV, :, :],
                            st16[:, 0:NDV, :, :], op=OP.mult,
                        )
                        nc.vector.tensor_reduce(
                            ocol(0, NDV), st16[:, 0:NDV, :, :],
                            axis=mybir.AxisListType.X, op=OP.add,
                        )
                    else:
                        scd = scrp.tile([P, NDV, 2, D], BF16, tag="scd")
                        nc.vector.tensor_tensor(
                            scd[:], st[:, 0:NDV, :, :], st[:, 0:NDV, :, :],
                            op=OP.mult,
                        )
                        nc.vector.tensor_reduce(
                            ocol(0, NDV), scd[:],
                            axis=mybir.AxisListType.X, op=OP.add,
                        )
                    # POOL share: batched square on gpsimd, DVE reduces
                    if NPL == 0:
                        pass
                    elif "xpool" in DBG:
                        nc.vector.memset(
                            f2t[b][:, co + 2 * NDV : co + 2 * (NDV + NPL)],
                            256.0,
                        )
                    else:
                        scp = scrp.tile([P, NPL, 2, D], BF16, tag="scp")
                        nc.gpsimd.tensor_tensor(
                            scp[:], st[:, NDV : NDV + NPL, :, :],
                            st[:, NDV : NDV + NPL, :, :], op=OP.mult,
                        )
                        nc.vector.tensor_reduce(
                            ocol(NDV, NDV + NPL), scp[:],
                            axis=mybir.AxisListType.X, op=OP.add,
                        )
                    # ACT share: per-chunk Square with accum_out
                    if "xact" in DBG:
                        nc.vector.memset(
                            f2t[b][:, co + 2 * (NDV + NPL) : co + 2 * GRP],
                            256.0,
                        )
                    else:
                        for u in range(NDV + NPL, GRP):
                            sca = scrp.tile([P, 2, D], BF16, tag="sca")
                            for t in range(2):
                                nc.scalar.activation(
                                    sca[:, t, :], st[:, u, t, :], AFT.Square,
                                    accum_out=f2t[b][:, co + 2 * u + t
                                                     : co + 2 * u + t + 1],
                                )

                for g in range(NGRP):
                    do_group(g)

                # ---- ft DMAs ----
                for b in range(NBLK):
                    ftA = ftp.tile([P, CPB * P], FP8, tag="ftA")
                    ftB = ftp.tile([P, CPB * P], FP8, tag="ftB")
                    bsl = slice(b * CPB * P, (b + 1) * CPB * P)
                    nc.sync.dma_start(ftA[:], fta_t.ap()[:, bsl])
                    nc.sync.dma_start(ftB[:], ftb_t.ap()[:, bsl])
                    fts[b] = (ftA, ftB)

                # ---- payload + allreduce ----
                payload = smallp.tile([C, D + 1], F32, tag="payload")
                nc.vector.tensor_copy(payload[:, 0:D], ps_sums[:])
                cnt_row = smallp.tile([1, C], F32, tag="cntrow")
                nc.vector.tensor_reduce(
                    cnt_row[:],
                    ps_cnt[:].rearrange("p (a c) -> p c a", c=C),
                    axis=mybir.AxisListType.X, op=OP.add,
                )
                ps_ct = ps_smallp.tile([C, 1], F32, tag="ps_small")
                nc.tensor.transpose(ps_ct[:], cnt_row[:], ident8[0:1, 0:1])
                nc.vector.tensor_copy(payload[:, D : D + 1], ps_ct[:])

                gsums = smallp.tile([C, D + 1], F32, tag="gsums")
                if "nocc" in DBG:
                    nc.vector.tensor_scalar_mul(gsums[:], payload[:],
                                                float(NCORES))
                else:
                    cc_in = dramp.tile([C, D + 1], F32, name=f"cci{rep}")
                    cc_out = dramp.tile([C, D + 1], F32, name=f"cco{rep}")
                    nc.gpsimd.dma_start(cc_in[:], payload[:])
                    nc.gpsimd.collective_compute(
                        "AllReduce", OP.add,
                        replica_groups=[list(range(NCORES))],
                        ins=[cc_in.opt()], outs=[cc_out.opt()],
                    )
                    nc.gpsimd.dma_start(gsums[:], cc_out[:])

                # ---- centers / weights ----
                counts = gsums[:, D : D + 1]
                cnt1 = smallp.tile([C, 1], F32, tag="cnt1")
                nc.vector.tensor_scalar_max(cnt1[:], counts, 1.0)
                recip = smallp.tile([C, 1], F32, tag="recip")
                nc.vector.reciprocal(recip[:], cnt1[:])
                centers = smallp.tile([C, D], F32, tag="centers")
                nc.vector.tensor_scalar(
                    centers[:], gsums[:, 0:D], recip[:], None, op0=OP.mult
                )
                ws = []
                for h in range(2):
                    ps_t = ps_smallp.tile([P, C], F32, tag="ps_small")
                    nc.tensor.transpose(
                        ps_t[:], centers[:, h * P : (h + 1) * P], ident8[:]
                    )
                    w = smallp.tile([P, C], FP8, tag=f"w{h}")
                    nc.vector.tensor_scalar_mul(w[:], ps_t[:], -2.0 * S)
                    ws.append(w)
                csq = smallp.tile([C, D], F32, tag="csq")
                nc.vector.tensor_tensor(csq[:], centers[:], centers[:],
                                        op=OP.mult)
                c2 = smallp.tile([C, 1], F32, tag="c2")
                nc.vector.tensor_reduce(
                    c2[:], csq[:], axis=mybir.AxisListType.X, op=OP.add
                )
                emptyb = smallp.tile([C, 1], F32, tag="emptyb")
                nc.vector.tensor_scalar(
                    emptyb[:], counts, 0.5, BIGC, op0=OP.is_lt, op1=OP.mult
                )
                c2s = smallp.tile([C, 1], F32, tag="c2s")
                nc.vector.scalar_tensor_tensor(
                    c2s[:], c2[:], S, emptyb[:], op0=OP.mult, op1=OP.add
                )
                ps_cr = ps_smallp.tile([1, C], F32, tag="ps_small")
                nc.tensor.transpose(ps_cr[:], c2s[:], ident8[:])
                c2r8 = smallp.tile([1, C], FP8, tag="c2r8")
                nc.vector.tensor_copy(c2r8[:], ps_cr[:])
                c2rep = smallp.tile([1, CPB * C], FP8, tag="c2rep")
                nc.vector.tensor_copy(c2rep[:, 0:C], c2r8[:])
                w_ = C
                while w_ < CPB * C:
                    nc.vector.tensor_copy(c2rep[:, w_ : 2 * w_],
                                          c2rep[:, 0:w_])
                    w_ *= 2

                # ---- pass 2 per block ----
                def do_block_p2(b):
                    ftA, ftB = fts[b]
                    ps_fc = ps_fcp.tile([P, CPB * C], F32, tag="ps_fc")
                    if "nop2" in DBG:
                        nc.vector.memset(ps_fc[:], 0.0)
                    else:
                        nc.tensor.matmul(
                            ps_fc[:], ones_r8[:], c2rep[:],
                            start=True, stop=False,
                        )
                        for q in range(CPB):
                            o = ps_fc[:, q * C : (q + 1) * C]
                            nc.tensor.matmul(
                                o, ftA[:, q * P : (q + 1) * P], ws[0][:],
                                start=False, stop=False,
                            )
                            nc.tensor.matmul(
                                o, ftB[:, q * P : (q + 1) * P], ws[1][:],
                                start=False, stop=(q == CPB - 1),
                            )
                    mn = blkp.tile([P, CPB], F32, tag="mn")
                    nc.vector.tensor_reduce(
                        mn[:],
                        ps_fc[:].rearrange("p (t c) -> p t c", c=C),
                        axis=mybir.AxisListType.X, op=OP.min,
                    )
                    return mn

                def do_block_tail(b, mn):
                    d2 = blkp.tile([P, CPB], F32, tag="d2")
                    nc.vector.scalar_tensor_tensor(
                        d2[:], mn[:], 1.0 / S, f2t[b][:],
                        op0=OP.mult, op1=OP.add,
                    )
                    dsc = blkp.tile([P, CPB], BF16, tag="dsc")
                    nc.scalar.activation(
                        dsc[:], d2[:], AFT.Sqrt,
                        accum_out=tot_cols[:, b : b + 1],
                    )

                for b in range(NBLK):
                    do_block_tail(b, do_block_p2(b))

                # ---- final total ----
                ps_tot = ps_smallp.tile([1, NBLK], F32, tag="ps_small")
                nc.tensor.matmul(
                    ps_tot[:], ones_c32[:], tot_cols[:],
                    start=True, stop=True,
                )
                res = smallp.tile([1, 1], F32, tag="res")
                nc.vector.tensor_reduce(
                    res[:], ps_tot[:], axis=mybir.AxisListType.X, op=OP.add
                )
                nc.sync.dma_start(out_t.ap()[rep : rep + 1, :], res[:])
                if "ser" in DBG:
                    res_prev = res

    nc.compile()
    return nc


_CACHE = {}


def _get_nc(R: int):
    if R not in _CACHE:
        _CACHE[R] = build_nc(R)
    return _CACHE[R]


GRP_H = GRP  # host-side group size; must match the kernel's GRP


def make_in_maps(features: np.ndarray, targets: np.ndarray,
                 ncores: int = NCORES):
    fp8np = mybir.dt.np(FP8)
    n = features.shape[0]
    r = n // ncores
    u = r // 256
    f8 = np.asarray(features, dtype=np.float32).astype(fp8np)
    tg = np.asarray(targets).astype(np.int64)
    in_maps = []
    for k in range(ncores):
        sl = slice(k * r, (k + 1) * r)
        f8c = f8[sl]
        tgc = tg[sl]
        # group-major: row (g, p) = [u0t0 | u0t1 | u1t0 | ...] for the 16
        # u-slots of group g -> contiguous 8KB per partition per group DMA
        natf = np.ascontiguousarray(
            f8c.reshape(u // GRP_H, GRP_H, 2, P, D)
            .transpose(0, 3, 1, 2, 4)
            .reshape((u // GRP_H) * P, GRP_H * 2 * D)
        )
        natb = None
        if B16:
            natb = np.ascontiguousarray(
                natf.astype(mybir.dt.np(BF16))
            )
        tgp = np.ascontiguousarray(
            tgc.reshape(u, 2, P).transpose(2, 0, 1).reshape(P, u * 2)
        ).astype(fp8np)
        ftc = np.ascontiguousarray(f8c.T)      # [256, r]
        im = {
            "natf": natf,
            "tgp": tgp,
            "fta": np.ascontiguousarray(ftc[0:P]),
            "ftb": np.ascontiguousarray(ftc[P:D]),
        }
        if natb is not None:
            im["natb"] = natb
        in_maps.append(im)
    return in_maps


def kernel(features, targets, **run_kwargs):
    features = np.asarray(features)
    targets = np.asarray(targets)
    n = features.shape[0]
    r = n // NCORES
    nc = _get_nc(r)
    in_maps = make_in_maps(features, targets)
    res = bass_utils.run_bass_kernel_spmd(
        nc, in_maps, core_ids=list(range(NCORES)), **run_kwargs
    )
    total = np.float64(0.0)
    for k in range(NCORES):
        total += np.float64(res.results[k]["partial"][0, 0])
    out = np.float32(total / n)
    if run_kwargs:
        return out, res
    return out


if __name__ == "__main__":
    nc = build_nc(65536)
    print("built OK")
